# revision 8
# baseline (speedup 1.0000x reference)
"""Bass/Trainium2 kernel for nn_Conv2d_mvm (bit-sliced analog-crossbar conv2d).

The reference's bit-slice / bit-stream decomposition is mathematically lossless:
  - weight slices recombine exactly to wq = round(w * 256)            (int)
  - input bit-streams recombine exactly to patches = im2col(round(x*256))
so the whole model is exactly:
    out_int = conv2d(xq, wq, pad=1)               (int32, exact)
    out     = clip(out_int >> 4, -2^15, 2^15-1) / 4096 + bias

Ranges (verified): |xq| <= ~1224, |wq| <= ~89, |out_int| < 2^22.
Therefore fp16 operands with fp32 PSUM accumulation compute out_int exactly.

Sharding: data-parallel over batch, 1 image per NeuronCore (8 cores).

Per-core device pipeline (v2):
  1. Parallel input DMAs: padded x fp32 [32,1156] in two column-halves,
     weights (3 tap-groups stacked on 128 partitions, fp32) + bias [128,193].
  2. Quantize on device: xq = round_half_even(x*256) via the 1.5*2^23
     magic-number trick (exact RNE, matches np.round), fp16 out.
  3. Contract-dim packing: 8 SBUF->SBUF DMAs build two [128,1088] tiles
     whose 32-partition blocks are tap-shifted copies of xq, so the 9-tap
     conv becomes 3 accumulating matmuls per spatial half
     (contract 128 / 128 / 32) instead of 9 matmuls of contract 32.
  4. Postprocess per half: clip in the fp32 domain fused with the int32
     convert (clip(v>>4) == clip(v, -2^19, 2^19-1) >> 4), arithmetic
     shift right 4 with fp32 convert-on-write (gpsimd), scale 1/4096 +
     per-channel bias (vector).
  5. Two output DMAs [64,512] (one per half, overlapped with compute).
"""

import numpy as np

import concourse.bass as bass
import concourse.mybir as mybir
import concourse.tile as tile
from concourse.bass_utils import run_bass_kernel_spmd

N_CORES = 8
MAGIC = 12582912.0  # 1.5 * 2**23: float add forces round-to-nearest-even int
CIN, COUT, H, W = 32, 64, 32, 32
PH, PW = H + 2, W + 2  # 34x34 padded
XCOLS = PH * PW        # 1156
NPIX = H * W           # 1024
RCOLS = 32 * PW        # 1088: replicated tile width (covers col 31*34+31)
RLEN = 31 * PW + W     # 1086: bytes actually needed per shifted copy

# tap t = di*3+dj reads padded pixel (oh+di, ow+dj) -> flat shift di*34+dj
TAPS = [(di, dj) for di in range(3) for dj in range(3)]
SHIFTS = [di * PW + dj for di, dj in TAPS]
GROUP_A = [0, 1, 2, 3]   # shifts 0,1,2,34
GROUP_B = [4, 5, 6, 7]   # shifts 35,36,68,69
TAP_C = 8                # shift 70 — reads xq directly (no copy)

# packed weight/bias buffer: [128, 193] fp32
#   cols   0- 63: lhsT_A (4 taps stacked on partitions)
#   cols  64-127: lhsT_B
#   cols 128-191: lhsT_C (rows 0-31)
#   col  192    : bias (rows 0-63)
WB_COLS = 193

_CACHE = {}


class SplitDrainTileContext(tile.TileContext):
    """TileContext whose end-of-kernel drain splits its semaphore waits
    across multiple single-wait Drain instructions.

    The stock TileContext attaches one wait per live proc (engine/DMA-lane
    semaphore) to a single SP Drain; TRN2 instructions only encode one
    sync-wait command, so walrus rejects the program ("Too many sync wait
    commands") whenever >1 procs are in flight at kernel end.
    """

    def _drain_and_barrier(self, tick_clock, wait_clock):
        from concourse.vector_clock import ScopedClock

        nc = self.nc
        drain_inst = nc.sync.drain()
        wait_clock.add_sem_waits(
            drain_inst.ins, ScopedClock({None: tick_clock.global_clock})
        )
        waits = list(drain_inst.ins.sync_info.on_wait)
        if len(waits) > 1:
            drain_inst.ins.sync_info = mybir.SyncInfo(
                on_wait=[waits[0]],
                on_update=list(drain_inst.ins.sync_info.on_update),
            )
            for w in waits[1:]:
                extra = nc.sync.drain()
                extra.ins.sync_info = mybir.SyncInfo(on_wait=[w], on_update=[])

        nc.all_engine_barrier()
        assert self.sems is not None
        popped = nc._tile_sem_poison_stack.pop()
        assert popped is self._sem_poison
        nc.clear_and_free_semaphores(list(self.sems.allocated().values()))
        nc.all_engine_barrier()


def _build_module():
    nc = bass.Bass("TRN2", target_bir_lowering=False, debug=False)

    x_d = nc.dram_tensor("xpad", [CIN, XCOLS], mybir.dt.float32,
                         kind="ExternalInput")
    wb_d = nc.dram_tensor("wb", [128, WB_COLS], mybir.dt.float32,
                          kind="ExternalInput")
    y_d = nc.dram_tensor("y", [COUT, NPIX], mybir.dt.float32,
                         kind="ExternalOutput")

    AL = mybir.AluOpType
    F32, F16, I32 = mybir.dt.float32, mybir.dt.float16, mybir.dt.int32
    XH = XCOLS // 2  # 578

    with SplitDrainTileContext(nc) as tc:
        from contextlib import ExitStack
        with ExitStack() as ctx:
            io = ctx.enter_context(tc.tile_pool(name="io", bufs=1))
            work = ctx.enter_context(tc.tile_pool(name="work", bufs=2))
            pp = ctx.enter_context(tc.tile_pool(name="psum", bufs=2, space="PSUM"))

            # --- input DMAs (parallel lanes) ---
            xt = io.tile([CIN, XCOLS], F32, tag="xt")
            nc.sync.dma_start(out=xt[:, 0:XH], in_=x_d[:, 0:XH])
            nc.sync.dma_start(out=xt[:, XH:XCOLS], in_=x_d[:, XH:XCOLS])
            wb = io.tile([128, WB_COLS], F32, tag="wb")
            nc.sync.dma_start(out=wb[:], in_=wb_d[:])

            # weights fp32 -> fp16 (exact: small integers)
            wt = io.tile([128, 192], F16, tag="wt")
            nc.vector.tensor_copy(wt[:], wb[:, 0:192])
            b_ap = wb[0:COUT, 192:193]

            # --- quantize: xq = RNE(x*256) as fp16 (exact, |xq| < 2048) ---
            q1 = io.tile([CIN, XCOLS], F32, tag="q1")
            nc.vector.tensor_scalar(out=q1[:, 0:XH], in0=xt[:, 0:XH],
                                    scalar1=256.0, scalar2=MAGIC,
                                    op0=AL.mult, op1=AL.add)
            nc.vector.tensor_scalar(out=q1[:, XH:XCOLS], in0=xt[:, XH:XCOLS],
                                    scalar1=256.0, scalar2=MAGIC,
                                    op0=AL.mult, op1=AL.add)
            xq = io.tile([CIN, XCOLS], F16, tag="xq")
            nc.vector.tensor_scalar(out=xq[:], in0=q1[:], scalar1=-MAGIC,
                                    scalar2=None, op0=AL.add)
            xq3 = xq[:].rearrange("p (r c) -> p r c", c=PW)

            # --- contract packing: tap-shifted copies of xq on 128 partitions
            rA = io.tile([128, RCOLS], F16, tag="rA")
            rB = io.tile([128, RCOLS], F16, tag="rB")
            for blk, t in enumerate(GROUP_A):
                nc.sync.dma_start(out=rA[32 * blk: 32 * blk + 32, 0:RLEN],
                                  in_=xq[:, SHIFTS[t]: SHIFTS[t] + RLEN])
            for blk, t in enumerate(GROUP_B):
                nc.sync.dma_start(out=rB[32 * blk: 32 * blk + 32, 0:RLEN],
                                  in_=xq[:, SHIFTS[t]: SHIFTS[t] + RLEN])
            rA3 = rA[:].rearrange("p (r c) -> p r c", c=PW)
            rB3 = rB[:].rearrange("p (r c) -> p r c", c=PW)

            for h in range(2):  # spatial halves: output rows [16h, 16h+16)
                ps = pp.tile([COUT, 512], F32, tag="ps")
                r0 = 16 * h
                nc.tensor.matmul(ps[:], wt[:, 0:64],
                                 rA3[:, r0:r0 + 16, 0:W], start=True, stop=False)
                nc.tensor.matmul(ps[:], wt[:, 64:128],
                                 rB3[:, r0:r0 + 16, 0:W], start=False, stop=False)
                nc.tensor.matmul(ps[:], wt[0:CIN, 128:192],
                                 xq3[:, 2 + r0:2 + r0 + 16, 2:2 + W],
                                 start=False, stop=True)

                # clip fused with int convert: clip(v>>4, +-2^15) ==
                # clip(v, -2^19, 2^19-1) >> 4 ; psum values are exact ints
                c32 = work.tile([COUT, 512], I32, tag="c32")
                nc.vector.tensor_scalar(out=c32[:], in0=ps[:],
                                        scalar1=float((1 << 19) - 1),
                                        scalar2=float(-(1 << 19)),
                                        op0=AL.min, op1=AL.max)
                # >>4 on gpsimd, converting to fp32 on write (exact)
                sf = work.tile([COUT, 512], F32, tag="sf")
                nc.gpsimd.tensor_scalar(out=sf[:], in0=c32[:], scalar1=4,
                                        scalar2=None, op0=AL.arith_shift_right)
                o = work.tile([COUT, 512], F32, tag="o")
                nc.vector.tensor_scalar(out=o[:], in0=sf[:],
                                        scalar1=1.0 / 4096.0, scalar2=b_ap,
                                        op0=AL.mult, op1=AL.add)
                nc.sync.dma_start(out=y_d[:, 512 * h: 512 * (h + 1)], in_=o[:])

    return nc


def get_nc():
    if "nc" not in _CACHE:
        _CACHE["nc"] = _build_module()
    return _CACHE["nc"]


def prep_in_maps(x, weight, bias):
    x = np.asarray(x, dtype=np.float32)
    weight = np.asarray(weight, dtype=np.float32)
    bias = np.asarray(bias, dtype=np.float32)

    # weight quantization (host): wq = round_half_even(w*256); |wq| <= ~89
    wq = np.round(weight * np.float32(256.0)).astype(np.float32)
    # per tap t: lhsT_t[ci, co] = wq[co, ci, di, dj]
    taps = wq.transpose(1, 2, 3, 0).reshape(CIN, 9, COUT)  # [ci, t, co]

    wb = np.zeros((128, WB_COLS), dtype=np.float32)
    for blk, t in enumerate(GROUP_A):
        wb[32 * blk: 32 * blk + 32, 0:64] = taps[:, t, :]
    for blk, t in enumerate(GROUP_B):
        wb[32 * blk: 32 * blk + 32, 64:128] = taps[:, t, :]
    wb[0:CIN, 128:192] = taps[:, TAP_C, :]
    wb[0:COUT, 192] = bias

    in_maps = []
    for c in range(N_CORES):
        xpad = np.pad(x[c], ((0, 0), (1, 1), (1, 1)))
        in_maps.append({
            "xpad": np.ascontiguousarray(xpad.reshape(CIN, XCOLS)),
            "wb": wb,
        })
    return in_maps


def run_spmd(in_maps, **kw):
    return run_bass_kernel_spmd(get_nc(), in_maps, list(range(N_CORES)), **kw)


def kernel(x, weight, bias):
    res = run_spmd(prep_in_maps(x, weight, bias))
    out = np.stack([r["y"].reshape(COUT, H, W) for r in res.results])
    return out.astype(np.float32)


# revision 15
# speedup vs baseline: 1.0514x; 1.0514x over previous
"""Bass/Trainium2 kernel for nn_Conv2d_mvm (bit-sliced analog-crossbar conv2d).

The reference's bit-slice / bit-stream decomposition is mathematically lossless:
  - weight slices recombine exactly to wq = round(w * 256)            (int)
  - input bit-streams recombine exactly to patches = im2col(round(x*256))
so the whole model is exactly:
    out_int = conv2d(xq, wq, pad=1)               (int32, exact)
    out     = clip(out_int >> 4, -2^15, 2^15-1) / 4096 + bias

Ranges (verified): |xq| <= ~1224, |wq| <= ~89, |out_int| < 2^22.
Therefore fp16 operands with fp32 PSUM accumulation compute out_int exactly.

Sharding: data-parallel over batch, 1 image per NeuronCore (8 cores).

Per-core device pipeline (v2):
  1. Parallel input DMAs: padded x fp32 [32,1156] in two column-halves,
     weights (3 tap-groups stacked on 128 partitions, fp32) + bias [128,193].
  2. Quantize on device: xq = round_half_even(x*256) via the 1.5*2^23
     magic-number trick (exact RNE, matches np.round), fp16 out.
  3. Contract-dim packing: 8 SBUF->SBUF DMAs build two [128,1088] tiles
     whose 32-partition blocks are tap-shifted copies of xq, so the 9-tap
     conv becomes 3 accumulating matmuls per spatial half
     (contract 128 / 128 / 32) instead of 9 matmuls of contract 32.
  4. Postprocess per half: clip in the fp32 domain fused with the int32
     convert (clip(v>>4) == clip(v, -2^19, 2^19-1) >> 4), arithmetic
     shift right 4 with fp32 convert-on-write (gpsimd), scale 1/4096 +
     per-channel bias (vector).
  5. Two output DMAs [64,512] (one per half, overlapped with compute).
"""

import numpy as np

import concourse.bass as bass
import concourse.mybir as mybir
import concourse.tile as tile
from concourse.bass_utils import run_bass_kernel_spmd

N_CORES = 8
MAGIC = 12582912.0  # 1.5 * 2**23: float add forces round-to-nearest-even int
CIN, COUT, H, W = 32, 64, 32, 32
PH, PW = H + 2, W + 2  # 34x34 padded
XCOLS = PH * PW        # 1156
NPIX = H * W           # 1024
RCOLS = 32 * PW        # 1088: replicated tile width (covers col 31*34+31)
RLEN = 31 * PW + W     # 1086: bytes actually needed per shifted copy

# tap t = di*3+dj reads padded pixel (oh+di, ow+dj) -> flat shift di*34+dj
TAPS = [(di, dj) for di in range(3) for dj in range(3)]
SHIFTS = [di * PW + dj for di, dj in TAPS]
GROUP_A = [0, 1, 2, 3]   # shifts 0,1,2,34
GROUP_B = [4, 5, 6, 7]   # shifts 35,36,68,69
TAP_C = 8                # shift 70 — reads xq directly (no copy)

# packed weight/bias buffer: [128, 193] fp32
#   cols   0- 63: lhsT_A (4 taps stacked on partitions)
#   cols  64-127: lhsT_B
#   cols 128-191: lhsT_C (rows 0-31)
#   col  192    : bias (rows 0-63)
WB_COLS = 193

_CACHE = {}


def _split_multi_waits(nc):
    """TRN2 instructions encode at most ONE sync-wait command; Tile happily
    attaches one wait per producer proc (DMA lane / engine semaphore) to a
    consumer, which walrus rejects ("Too many sync wait commands").  Hoist
    the extra waits onto fresh single-wait NoOps inserted just before the
    instruction on the same engine (engine queues are in-order, so the
    semantics are identical)."""
    k = 0
    for f in nc.m.functions:
        for bb in f.blocks:
            insts = bb.instructions
            i = 0
            while i < len(insts):
                inst = insts[i]
                si = inst.sync_info
                if si is not None and len(si.on_wait) > 1:
                    waits = list(si.on_wait)
                    for w in waits[:-1]:
                        nop = mybir.InstNoOp(name=f"splitw_{k}", ins=[], outs=[])
                        k += 1
                        nop.engine = inst.engine
                        nop.sync_info = mybir.SyncInfo(on_wait=[w], on_update=[])
                        nc.register_instruction(nop)
                        insts.insert(i, nop)
                        i += 1
                    inst.sync_info = mybir.SyncInfo(
                        on_wait=[waits[-1]], on_update=list(si.on_update))
                i += 1
    return nc


def _build_module():
    nc = bass.Bass("TRN2", target_bir_lowering=False, debug=False)

    x_d = nc.dram_tensor("xpad", [CIN, XCOLS], mybir.dt.float32,
                         kind="ExternalInput")
    wb_d = nc.dram_tensor("wb", [128, WB_COLS], mybir.dt.float32,
                          kind="ExternalInput")
    y_d = nc.dram_tensor("y", [COUT, NPIX], mybir.dt.float32,
                         kind="ExternalOutput")

    AL = mybir.AluOpType
    F32, F16, I32 = mybir.dt.float32, mybir.dt.float16, mybir.dt.int32
    XH = XCOLS // 2  # 578

    with tile.TileContext(nc) as tc:
        from contextlib import ExitStack
        with ExitStack() as ctx:
            io = ctx.enter_context(tc.tile_pool(name="io", bufs=1))
            work = ctx.enter_context(tc.tile_pool(name="work", bufs=2))
            pp = ctx.enter_context(tc.tile_pool(name="psum", bufs=2, space="PSUM"))

            # --- input DMAs (parallel lanes) ---
            xt = io.tile([CIN, XCOLS], F32, tag="xt")
            nc.sync.dma_start(out=xt[:, 0:XH], in_=x_d[:, 0:XH])
            nc.sync.dma_start(out=xt[:, XH:XCOLS], in_=x_d[:, XH:XCOLS])
            wb = io.tile([128, WB_COLS], F32, tag="wb")
            nc.sync.dma_start(out=wb[:], in_=wb_d[:])

            # weights fp32 -> fp16 (exact: small integers)
            wt = io.tile([128, 192], F16, tag="wt")
            nc.vector.tensor_copy(wt[:], wb[:, 0:192])
            b_ap = wb[0:COUT, 192:193]

            # --- quantize: xq = RNE(x*256) as fp16 (exact, |xq| < 2048) ---
            q1 = io.tile([CIN, XCOLS], F32, tag="q1")
            nc.vector.tensor_scalar(out=q1[:, 0:XH], in0=xt[:, 0:XH],
                                    scalar1=256.0, scalar2=MAGIC,
                                    op0=AL.mult, op1=AL.add)
            nc.vector.tensor_scalar(out=q1[:, XH:XCOLS], in0=xt[:, XH:XCOLS],
                                    scalar1=256.0, scalar2=MAGIC,
                                    op0=AL.mult, op1=AL.add)
            xq = io.tile([CIN, XCOLS], F16, tag="xq")
            nc.vector.tensor_scalar(out=xq[:], in0=q1[:], scalar1=-MAGIC,
                                    scalar2=None, op0=AL.add)
            xq3 = xq[:].rearrange("p (r c) -> p r c", c=PW)

            # --- contract packing: tap-shifted copies of xq on 128 partitions
            rA = io.tile([128, RCOLS], F16, tag="rA")
            rB = io.tile([128, RCOLS], F16, tag="rB")
            for blk, t in enumerate(GROUP_A):
                nc.sync.dma_start(out=rA[32 * blk: 32 * blk + 32, 0:RLEN],
                                  in_=xq[:, SHIFTS[t]: SHIFTS[t] + RLEN])
            for blk, t in enumerate(GROUP_B):
                nc.sync.dma_start(out=rB[32 * blk: 32 * blk + 32, 0:RLEN],
                                  in_=xq[:, SHIFTS[t]: SHIFTS[t] + RLEN])
            rA3 = rA[:].rearrange("p (r c) -> p r c", c=PW)
            rB3 = rB[:].rearrange("p (r c) -> p r c", c=PW)

            for h in range(2):  # spatial halves: output rows [16h, 16h+16)
                ps = pp.tile([COUT, 512], F32, tag="ps")
                r0 = 16 * h
                nc.tensor.matmul(ps[:], wt[:, 0:64],
                                 rA3[:, r0:r0 + 16, 0:W], start=True, stop=False)
                nc.tensor.matmul(ps[:], wt[:, 64:128],
                                 rB3[:, r0:r0 + 16, 0:W], start=False, stop=False)
                nc.tensor.matmul(ps[:], wt[0:CIN, 128:192],
                                 xq3[:, 2 + r0:2 + r0 + 16, 2:2 + W],
                                 start=False, stop=True)

                # clip fused with int convert: clip(v>>4, +-2^15) ==
                # clip(v, -2^19, 2^19-1) >> 4 ; psum values are exact ints
                c32 = work.tile([COUT, 512], I32, tag="c32")
                nc.vector.tensor_scalar(out=c32[:], in0=ps[:],
                                        scalar1=float((1 << 19) - 1),
                                        scalar2=float(-(1 << 19)),
                                        op0=AL.min, op1=AL.max)
                # >>4 (bitwise ops cannot cast, stay int32)
                sf = work.tile([COUT, 512], I32, tag="sf")
                nc.vector.tensor_scalar(out=sf[:], in0=c32[:], scalar1=4,
                                        scalar2=None, op0=AL.arith_shift_right)
                # scalar engine: int32 -> fp32, q/4096 + bias (both exact)
                o = work.tile([COUT, 512], F32, tag="o")
                nc.scalar.activation(o[:], sf[:],
                                     mybir.ActivationFunctionType.Identity,
                                     bias=b_ap, scale=1.0 / 4096.0)
                nc.sync.dma_start(out=y_d[:, 512 * h: 512 * (h + 1)], in_=o[:])

    return _split_multi_waits(nc)


def get_nc():
    if "nc" not in _CACHE:
        _CACHE["nc"] = _build_module()
    return _CACHE["nc"]


def prep_in_maps(x, weight, bias):
    x = np.asarray(x, dtype=np.float32)
    weight = np.asarray(weight, dtype=np.float32)
    bias = np.asarray(bias, dtype=np.float32)

    # weight quantization (host): wq = round_half_even(w*256); |wq| <= ~89
    wq = np.round(weight * np.float32(256.0)).astype(np.float32)
    # per tap t: lhsT_t[ci, co] = wq[co, ci, di, dj]
    taps = wq.transpose(1, 2, 3, 0).reshape(CIN, 9, COUT)  # [ci, t, co]

    wb = np.zeros((128, WB_COLS), dtype=np.float32)
    for blk, t in enumerate(GROUP_A):
        wb[32 * blk: 32 * blk + 32, 0:64] = taps[:, t, :]
    for blk, t in enumerate(GROUP_B):
        wb[32 * blk: 32 * blk + 32, 64:128] = taps[:, t, :]
    wb[0:CIN, 128:192] = taps[:, TAP_C, :]
    wb[0:COUT, 192] = bias

    in_maps = []
    for c in range(N_CORES):
        xpad = np.pad(x[c], ((0, 0), (1, 1), (1, 1)))
        in_maps.append({
            "xpad": np.ascontiguousarray(xpad.reshape(CIN, XCOLS)),
            "wb": wb,
        })
    return in_maps


def run_spmd(in_maps, **kw):
    return run_bass_kernel_spmd(get_nc(), in_maps, list(range(N_CORES)), **kw)


def kernel(x, weight, bias):
    res = run_spmd(prep_in_maps(x, weight, bias))
    out = np.stack([r["y"].reshape(COUT, H, W) for r in res.results])
    return out.astype(np.float32)


# revision 17
# speedup vs baseline: 1.1309x; 1.0756x over previous
"""Bass/Trainium2 kernel for nn_Conv2d_mvm (bit-sliced analog-crossbar conv2d).

The reference's bit-slice / bit-stream decomposition is mathematically lossless:
  - weight slices recombine exactly to wq = round(w * 256)            (int)
  - input bit-streams recombine exactly to patches = im2col(round(x*256))
so the whole model is exactly:
    out_int = conv2d(xq, wq, pad=1)               (int32, exact)
    out     = clip(out_int >> 4, -2^15, 2^15-1) / 4096 + bias

Ranges (verified): |xq| <= ~1224, |wq| <= ~89, |out_int| < 2^22.
Therefore fp16 operands with fp32 PSUM accumulation compute out_int exactly.

Sharding: data-parallel over batch, 1 image per NeuronCore (8 cores).

Per-core device pipeline (v3):
  1. Parallel input DMAs on both HWDGE queues (SP + ACT): padded x fp32
     [32,1156] in two column-halves, packed weights+bias [96,193] fp32.
  2. Quantize on device: xq = round_half_even(x*256) via the 1.5*2^23
     magic-number trick (exact RNE, matches np.round), fp16 out.
  3. Contract-dim packing, one DMA per kernel row r: an overlapping
     access pattern replicates xq three ways (shifts 34r+{0,1,2}) into a
     [96,1088] tile, so the 9-tap conv becomes 3 accumulating matmuls of
     contract 96 per spatial half.
  4. Postprocess per half: clip fused with the fp32->int32 convert
     (clip(v>>4) == clip(v, -2^19, 2^19-1) >> 4), arithmetic shift right
     4 (vector), then scale 1/4096 + per-channel bias on the scalar
     engine (int32 read, exact).
  5. Two output DMAs [64,512] (one per half, on separate queues).
"""

import numpy as np

import concourse.bass as bass
import concourse.mybir as mybir
import concourse.tile as tile
from concourse.bass_utils import run_bass_kernel_spmd

N_CORES = 8
MAGIC = 12582912.0  # 1.5 * 2**23: float add forces round-to-nearest-even int
CIN, COUT, H, W = 32, 64, 32, 32
PH, PW = H + 2, W + 2  # 34x34 padded
XCOLS = PH * PW        # 1156
NPIX = H * W           # 1024
RCOLS = 32 * PW        # 1088: replicated tile width
RLEN = 31 * PW + W     # 1086: columns actually needed per shifted copy

# packed weight/bias buffer [96, 193] fp32:
#   col block 64r..64r+64 : lhsT of kernel-row group r (taps (r,0..2)
#   stacked on partition blocks 32k), rows 0-95
#   col 192: bias (rows 0-63)
WB_COLS = 193

_CACHE = {}


def _split_multi_waits(nc):
    """TRN2 instructions encode at most ONE sync-wait command; Tile happily
    attaches one wait per producer proc (DMA lane / engine semaphore) to a
    consumer, which walrus rejects ("Too many sync wait commands").  Hoist
    the extra waits onto fresh single-wait NoOps inserted just before the
    instruction on the same engine (engine queues are in-order, so the
    semantics are identical)."""
    k = 0
    for f in nc.m.functions:
        for bb in f.blocks:
            insts = bb.instructions
            i = 0
            while i < len(insts):
                inst = insts[i]
                si = inst.sync_info
                if si is not None and len(si.on_wait) > 1:
                    waits = list(si.on_wait)
                    for w in waits[:-1]:
                        nop = mybir.InstNoOp(name=f"splitw_{k}", ins=[], outs=[])
                        k += 1
                        nop.engine = inst.engine
                        nop.sync_info = mybir.SyncInfo(on_wait=[w], on_update=[])
                        nc.register_instruction(nop)
                        insts.insert(i, nop)
                        i += 1
                    inst.sync_info = mybir.SyncInfo(
                        on_wait=[waits[-1]], on_update=list(si.on_update))
                i += 1
    return nc


def _repl_aps(xq, rG, shift):
    """APs for one replication DMA: 3 overlapping shifted copies of xq
    (cols shift+k, k=0..2) -> partition blocks 32k of rG."""
    a = xq[:, shift: shift + RLEN]
    src = bass.AP(a.tensor, a.offset, [list(a.ap[0]), [1, 3], [1, RLEN]])
    d = rG[:, 0:RLEN]
    dst = bass.AP(d.tensor, d.offset, [[RCOLS, 32], [32 * RCOLS, 3], [1, RLEN]])
    return dst, src


def _build_module():
    nc = bass.Bass("TRN2", target_bir_lowering=False, debug=False)

    x_d = nc.dram_tensor("xpad", [CIN, XCOLS], mybir.dt.float32,
                         kind="ExternalInput")
    wb_d = nc.dram_tensor("wb", [96, WB_COLS], mybir.dt.float32,
                          kind="ExternalInput")
    y_d = nc.dram_tensor("y", [COUT, NPIX], mybir.dt.float32,
                         kind="ExternalOutput")

    AL = mybir.AluOpType
    F32, F16, I32 = mybir.dt.float32, mybir.dt.float16, mybir.dt.int32
    XH = XCOLS // 2  # 578

    with tile.TileContext(nc) as tc:
        from contextlib import ExitStack
        with ExitStack() as ctx:
            io = ctx.enter_context(tc.tile_pool(name="io", bufs=1))
            work = ctx.enter_context(tc.tile_pool(name="work", bufs=2))
            pp = ctx.enter_context(tc.tile_pool(name="psum", bufs=2, space="PSUM"))

            # --- input DMAs: split across the two HWDGE queues (SP, ACT) ---
            xt = io.tile([CIN, XCOLS], F32, tag="xt")
            nc.sync.dma_start(out=xt[:, 0:XH], in_=x_d[:, 0:XH])
            nc.scalar.dma_start(out=xt[:, XH:XCOLS], in_=x_d[:, XH:XCOLS])
            wb = io.tile([96, WB_COLS], F32, tag="wb")
            nc.sync.dma_start(out=wb[:], in_=wb_d[:])

            # weights fp32 -> fp16 (exact: small integers)
            wt = io.tile([96, 192], F16, tag="wt")
            nc.vector.tensor_copy(wt[:], wb[:, 0:192])
            b_ap = wb[0:COUT, 192:193]

            # --- quantize: xq = RNE(x*256) as fp16 (exact, |xq| < 2048) ---
            q1 = io.tile([CIN, XCOLS], F32, tag="q1")
            nc.vector.tensor_scalar(out=q1[:, 0:XH], in0=xt[:, 0:XH],
                                    scalar1=256.0, scalar2=MAGIC,
                                    op0=AL.mult, op1=AL.add)
            nc.vector.tensor_scalar(out=q1[:, XH:XCOLS], in0=xt[:, XH:XCOLS],
                                    scalar1=256.0, scalar2=MAGIC,
                                    op0=AL.mult, op1=AL.add)
            xq = io.tile([CIN, XCOLS], F16, tag="xq")
            nc.vector.tensor_scalar(out=xq[:], in0=q1[:], scalar1=-MAGIC,
                                    scalar2=None, op0=AL.add)

            # --- one replication DMA per kernel row (3 shifted copies each)
            rG = [io.tile([96, RCOLS], F16, tag=f"rG{r}", name=f"rG{r}")
                  for r in range(3)]
            dma_eng = [nc.sync, nc.scalar, nc.sync]
            for r in range(3):
                dst, src = _repl_aps(xq, rG[r], PW * r)
                dma_eng[r].dma_start(out=dst, in_=src)
            rG3 = [t[:].rearrange("p (r c) -> p r c", c=PW) for t in rG]

            out_eng = [nc.scalar, nc.sync]
            for h in range(2):  # spatial halves: output rows [16h, 16h+16)
                ps = pp.tile([COUT, 512], F32, tag="ps")
                r0 = 16 * h
                for r in range(3):
                    nc.tensor.matmul(ps[:], wt[:, 64 * r: 64 * r + 64],
                                     rG3[r][:, r0:r0 + 16, 0:W],
                                     start=(r == 0), stop=(r == 2))

                # clip fused with int convert: clip(v>>4, +-2^15) ==
                # clip(v, -2^19, 2^19-1) >> 4 ; psum values are exact ints
                c32 = work.tile([COUT, 512], I32, tag="c32")
                nc.vector.tensor_scalar(out=c32[:], in0=ps[:],
                                        scalar1=float((1 << 19) - 1),
                                        scalar2=float(-(1 << 19)),
                                        op0=AL.min, op1=AL.max)
                sf = work.tile([COUT, 512], I32, tag="sf")
                nc.vector.tensor_scalar(out=sf[:], in0=c32[:], scalar1=4,
                                        scalar2=None, op0=AL.arith_shift_right)
                # scalar engine: int32 -> fp32, q/4096 + bias (both exact)
                o = work.tile([COUT, 512], F32, tag="o")
                nc.scalar.activation(o[:], sf[:],
                                     mybir.ActivationFunctionType.Identity,
                                     bias=b_ap, scale=1.0 / 4096.0)
                out_eng[h].dma_start(out=y_d[:, 512 * h: 512 * (h + 1)],
                                     in_=o[:])

    return _split_multi_waits(nc)


def get_nc():
    if "nc" not in _CACHE:
        _CACHE["nc"] = _build_module()
    return _CACHE["nc"]


def prep_in_maps(x, weight, bias):
    x = np.asarray(x, dtype=np.float32)
    weight = np.asarray(weight, dtype=np.float32)
    bias = np.asarray(bias, dtype=np.float32)

    # weight quantization (host): wq = round_half_even(w*256); |wq| <= ~89
    wq = np.round(weight * np.float32(256.0)).astype(np.float32)
    # per tap (di,dj): lhsT[ci, co] = wq[co, ci, di, dj]
    taps = wq.transpose(1, 2, 3, 0).reshape(CIN, 9, COUT)  # [ci, t, co]

    wb = np.zeros((96, WB_COLS), dtype=np.float32)
    for r in range(3):
        for k in range(3):
            wb[32 * k: 32 * k + 32, 64 * r: 64 * r + 64] = taps[:, 3 * r + k, :]
    wb[0:COUT, 192] = bias

    in_maps = []
    for c in range(N_CORES):
        xpad = np.pad(x[c], ((0, 0), (1, 1), (1, 1)))
        in_maps.append({
            "xpad": np.ascontiguousarray(xpad.reshape(CIN, XCOLS)),
            "wb": wb,
        })
    return in_maps


def run_spmd(in_maps, **kw):
    return run_bass_kernel_spmd(get_nc(), in_maps, list(range(N_CORES)), **kw)


def kernel(x, weight, bias):
    res = run_spmd(prep_in_maps(x, weight, bias))
    out = np.stack([r["y"].reshape(COUT, H, W) for r in res.results])
    return out.astype(np.float32)


# revision 18
# speedup vs baseline: 1.1445x; 1.0121x over previous
"""Bass/Trainium2 kernel for nn_Conv2d_mvm (bit-sliced analog-crossbar conv2d).

The reference's bit-slice / bit-stream decomposition is mathematically lossless:
  - weight slices recombine exactly to wq = round(w * 256)            (int)
  - input bit-streams recombine exactly to patches = im2col(round(x*256))
so the whole model is exactly:
    out_int = conv2d(xq, wq, pad=1)               (int32, exact)
    out     = clip(out_int >> 4, -2^15, 2^15-1) / 4096 + bias

Ranges (verified): |xq| <= ~1224, |wq| <= ~89, |out_int| < 2^22.
Therefore fp16 operands with fp32 PSUM accumulation compute out_int exactly.

Sharding: data-parallel over batch, 1 image per NeuronCore (8 cores).

Per-core device pipeline (v3):
  1. Parallel input DMAs on both HWDGE queues (SP + ACT): padded x fp32
     [32,1156] in two column-halves, packed weights+bias [96,193] fp32.
  2. Quantize on device: xq = round_half_even(x*256) via the 1.5*2^23
     magic-number trick (exact RNE, matches np.round), fp16 out.
  3. Contract-dim packing, one DMA per kernel row r: an overlapping
     access pattern replicates xq three ways (shifts 34r+{0,1,2}) into a
     [96,1088] tile, so the 9-tap conv becomes 3 accumulating matmuls of
     contract 96 per spatial half.
  4. Postprocess per half: clip fused with the fp32->int32 convert
     (clip(v>>4) == clip(v, -2^19, 2^19-1) >> 4), arithmetic shift right
     4 (vector), then scale 1/4096 + per-channel bias on the scalar
     engine (int32 read, exact).
  5. Two output DMAs [64,512] (one per half, on separate queues).
"""

import numpy as np

import concourse.bass as bass
import concourse.mybir as mybir
import concourse.tile as tile
from concourse.bass_utils import run_bass_kernel_spmd

N_CORES = 8
MAGIC = 12582912.0  # 1.5 * 2**23: float add forces round-to-nearest-even int
CIN, COUT, H, W = 32, 64, 32, 32
PH, PW = H + 2, W + 2  # 34x34 padded
XCOLS = PH * PW        # 1156
NPIX = H * W           # 1024
RCOLS = 32 * PW        # 1088: replicated tile width
RLEN = 31 * PW + W     # 1086: columns actually needed per shifted copy

# packed weight/bias buffer [96, 193] fp32:
#   col block 64r..64r+64 : lhsT of kernel-row group r (taps (r,0..2)
#   stacked on partition blocks 32k), rows 0-95
#   col 192: bias (rows 0-63)
WB_COLS = 193

_CACHE = {}


def _split_multi_waits(nc):
    """TRN2 instructions encode at most ONE sync-wait command; Tile happily
    attaches one wait per producer proc (DMA lane / engine semaphore) to a
    consumer, which walrus rejects ("Too many sync wait commands").  Hoist
    the extra waits onto fresh single-wait NoOps inserted just before the
    instruction on the same engine (engine queues are in-order, so the
    semantics are identical)."""
    k = 0
    for f in nc.m.functions:
        for bb in f.blocks:
            insts = bb.instructions
            i = 0
            while i < len(insts):
                inst = insts[i]
                si = inst.sync_info
                if si is not None and len(si.on_wait) > 1:
                    waits = list(si.on_wait)
                    for w in waits[:-1]:
                        nop = mybir.InstNoOp(name=f"splitw_{k}", ins=[], outs=[])
                        k += 1
                        nop.engine = inst.engine
                        nop.sync_info = mybir.SyncInfo(on_wait=[w], on_update=[])
                        nc.register_instruction(nop)
                        insts.insert(i, nop)
                        i += 1
                    inst.sync_info = mybir.SyncInfo(
                        on_wait=[waits[-1]], on_update=list(si.on_update))
                i += 1
    return nc


def _repl_aps(xq, rG, shift):
    """APs for one replication DMA: 3 overlapping shifted copies of xq
    (cols shift+k, k=0..2) -> partition blocks 32k of rG."""
    a = xq[:, shift: shift + RLEN]
    src = bass.AP(a.tensor, a.offset, [list(a.ap[0]), [1, 3], [1, RLEN]])
    d = rG[:, 0:RLEN]
    dst = bass.AP(d.tensor, d.offset, [[RCOLS, 32], [32 * RCOLS, 3], [1, RLEN]])
    return dst, src


def _build_module():
    nc = bass.Bass("TRN2", target_bir_lowering=False, debug=False)

    x_d = nc.dram_tensor("xpad", [CIN, XCOLS], mybir.dt.float32,
                         kind="ExternalInput")
    wb_d = nc.dram_tensor("wb", [96, WB_COLS], mybir.dt.float32,
                          kind="ExternalInput")
    y_d = nc.dram_tensor("y", [COUT, NPIX], mybir.dt.float32,
                         kind="ExternalOutput")

    AL = mybir.AluOpType
    F32, F16, I32 = mybir.dt.float32, mybir.dt.float16, mybir.dt.int32
    XH = XCOLS // 2  # 578

    with tile.TileContext(nc) as tc:
        from contextlib import ExitStack
        with ExitStack() as ctx:
            io = ctx.enter_context(tc.tile_pool(name="io", bufs=1))
            work = ctx.enter_context(tc.tile_pool(name="work", bufs=2))
            pp = ctx.enter_context(tc.tile_pool(name="psum", bufs=2, space="PSUM"))

            # --- input DMAs: split across the two HWDGE queues (SP, ACT) ---
            xt = io.tile([CIN, XCOLS], F32, tag="xt")
            nc.sync.dma_start(out=xt[:, 0:XH], in_=x_d[:, 0:XH])
            nc.scalar.dma_start(out=xt[:, XH:XCOLS], in_=x_d[:, XH:XCOLS])
            wb = io.tile([96, WB_COLS], F32, tag="wb")
            nc.sync.dma_start(out=wb[:], in_=wb_d[:])

            # weights fp32 -> fp16 (exact: small integers)
            wt = io.tile([96, 192], F16, tag="wt")
            nc.vector.tensor_copy(wt[:], wb[:, 0:192])
            b_ap = wb[0:COUT, 192:193]

            # --- quantize: xq = RNE(x*256) as fp16 (exact, |xq| < 2048) ---
            q1 = io.tile([CIN, XCOLS], F32, tag="q1")
            nc.vector.tensor_scalar(out=q1[:, 0:XH], in0=xt[:, 0:XH],
                                    scalar1=256.0, scalar2=MAGIC,
                                    op0=AL.mult, op1=AL.add)
            nc.vector.tensor_scalar(out=q1[:, XH:XCOLS], in0=xt[:, XH:XCOLS],
                                    scalar1=256.0, scalar2=MAGIC,
                                    op0=AL.mult, op1=AL.add)
            xq = io.tile([CIN, XCOLS], F16, tag="xq")
            nc.vector.tensor_scalar(out=xq[:], in0=q1[:], scalar1=-MAGIC,
                                    scalar2=None, op0=AL.add)

            # --- one replication DMA per kernel row (3 shifted copies each)
            rG = [io.tile([96, RCOLS], F16, tag=f"rG{r}", name=f"rG{r}")
                  for r in range(3)]
            dma_eng = [nc.sync, nc.scalar, nc.sync]
            for r in range(3):
                # The overlapping dst AP is invisible to Tile's range tracker
                # (it only sees partitions 0-31); a prior full-tile memset
                # anchors the dep chain: DMA after memset (WAW overlap),
                # matmuls after DMA (read overlap in partitions 0-31, and the
                # DMA completion semaphore covers the whole transfer).
                nc.gpsimd.memset(rG[r][:], 0.0)
                dst, src = _repl_aps(xq, rG[r], PW * r)
                dma_eng[r].dma_start(out=dst, in_=src)
            rG3 = [t[:].rearrange("p (r c) -> p r c", c=PW) for t in rG]

            out_eng = [nc.scalar, nc.sync]
            for h in range(2):  # spatial halves: output rows [16h, 16h+16)
                ps = pp.tile([COUT, 512], F32, tag="ps")
                r0 = 16 * h
                for r in range(3):
                    nc.tensor.matmul(ps[:], wt[:, 64 * r: 64 * r + 64],
                                     rG3[r][:, r0:r0 + 16, 0:W],
                                     start=(r == 0), stop=(r == 2))

                # clip fused with int convert: clip(v>>4, +-2^15) ==
                # clip(v, -2^19, 2^19-1) >> 4 ; psum values are exact ints
                c32 = work.tile([COUT, 512], I32, tag="c32")
                nc.vector.tensor_scalar(out=c32[:], in0=ps[:],
                                        scalar1=float((1 << 19) - 1),
                                        scalar2=float(-(1 << 19)),
                                        op0=AL.min, op1=AL.max)
                sf = work.tile([COUT, 512], I32, tag="sf")
                nc.vector.tensor_scalar(out=sf[:], in0=c32[:], scalar1=4,
                                        scalar2=None, op0=AL.arith_shift_right)
                # scalar engine: int32 -> fp32, q/4096 + bias (both exact)
                o = work.tile([COUT, 512], F32, tag="o")
                nc.scalar.activation(o[:], sf[:],
                                     mybir.ActivationFunctionType.Identity,
                                     bias=b_ap, scale=1.0 / 4096.0)
                out_eng[h].dma_start(out=y_d[:, 512 * h: 512 * (h + 1)],
                                     in_=o[:])

    return _split_multi_waits(nc)


def get_nc():
    if "nc" not in _CACHE:
        _CACHE["nc"] = _build_module()
    return _CACHE["nc"]


def prep_in_maps(x, weight, bias):
    x = np.asarray(x, dtype=np.float32)
    weight = np.asarray(weight, dtype=np.float32)
    bias = np.asarray(bias, dtype=np.float32)

    # weight quantization (host): wq = round_half_even(w*256); |wq| <= ~89
    wq = np.round(weight * np.float32(256.0)).astype(np.float32)
    # per tap (di,dj): lhsT[ci, co] = wq[co, ci, di, dj]
    taps = wq.transpose(1, 2, 3, 0).reshape(CIN, 9, COUT)  # [ci, t, co]

    wb = np.zeros((96, WB_COLS), dtype=np.float32)
    for r in range(3):
        for k in range(3):
            wb[32 * k: 32 * k + 32, 64 * r: 64 * r + 64] = taps[:, 3 * r + k, :]
    wb[0:COUT, 192] = bias

    in_maps = []
    for c in range(N_CORES):
        xpad = np.pad(x[c], ((0, 0), (1, 1), (1, 1)))
        in_maps.append({
            "xpad": np.ascontiguousarray(xpad.reshape(CIN, XCOLS)),
            "wb": wb,
        })
    return in_maps


def run_spmd(in_maps, **kw):
    return run_bass_kernel_spmd(get_nc(), in_maps, list(range(N_CORES)), **kw)


def kernel(x, weight, bias):
    res = run_spmd(prep_in_maps(x, weight, bias))
    out = np.stack([r["y"].reshape(COUT, H, W) for r in res.results])
    return out.astype(np.float32)
